# revision 12
# baseline (speedup 1.0000x reference)
"""Multi-head self-attention (RoPE, causal) on 8 Trainium2 NeuronCores.

Sharding: tensor-parallel over heads. Each core owns 2 of 16 heads:
  - QKV projections column-sharded (each core computes its 128 features)
  - attention per (batch, head) fully on-core; scores kept transposed
    [tk, tq] so softmax needs no PE transposes; exp on ACT (both heads in
    one instruction via a strided AP); denominator via a ones-row in V;
    1/denom via ACT ln/exp (batched per half-batch to avoid table
    reloads); broadcast via gpsimd partition_broadcast
  - two AllToAlls per batch (strided token blocks, halves) switch
    head-sharding to token-sharding; output projection per half-batch,
    overlapped with the next batch's attention
  - phase-1 chunks for batch b+1 and oproj for batch b-1 are interleaved
    into batch b's attention loop so the PE never starves

dtypes: bf16 for all matmul operands; fp32 PSUM and softmax statistics.
"""

import numpy as np
import ml_dtypes

import concourse.bacc as bacc
import concourse.mybir as mybir
import concourse.tile as tile
from concourse import bass_utils

F32 = mybir.dt.float32
BF16 = mybir.dt.bfloat16
FP8 = mybir.dt.float8e4
W_SCALE = 32.0

B, T, D = 4, 2048, 1024
H, DH = 16, 64
N_CORES = 8
HPC = H // N_CORES            # heads per core = 2
EC = HPC * DH                 # feature slice per core = 128
NT = B * T                    # 8192 tokens
TPC = NT // N_CORES           # 1024 tokens per core
THETA = 10000.0
NBB = T // 128                # 16 tk blocks per batch

_CACHE = {}
last_results = None


def _build_program():
    nc = bacc.Bacc("TRN2", debug=False, target_bir_lowering=False,
                   num_devices=N_CORES)

    xt_d = nc.dram_tensor("xt", [128, 8, NT], BF16, kind="ExternalInput")
    wq_d = nc.dram_tensor("wq", [128, 8, EC], BF16, kind="ExternalInput")
    wk_d = nc.dram_tensor("wk", [128, 8, EC], BF16, kind="ExternalInput")
    wv_d = nc.dram_tensor("wv", [128, 8, EC], BF16, kind="ExternalInput")
    wo_d = nc.dram_tensor("wo", [128, 8, D], BF16, kind="ExternalInput")
    cos_d = nc.dram_tensor("cosb", [128, T], F32, kind="ExternalInput")
    sin_d = nc.dram_tensor("sinb", [128, T], F32, kind="ExternalInput")
    rotm_d = nc.dram_tensor("rotm", [128, 128], BF16, kind="ExternalInput")
    tri_d = nc.dram_tensor("trimask", [128, 128], BF16, kind="ExternalInput")
    id_d = nc.dram_tensor("identb", [128, 128], BF16, kind="ExternalInput")
    y_d = nc.dram_tensor("y", [TPC, D], F32, kind="ExternalOutput")

    with tile.TileContext(nc) as tc:
        with (
            tc.tile_pool(name="consts", bufs=1) as consts,
            tc.tile_pool(name="big", bufs=1) as big,
            tc.tile_pool(name="xp", bufs=2) as xp,
            tc.tile_pool(name="stage", bufs=2) as stage,
            tc.tile_pool(name="expp", bufs=6) as expp,
            tc.tile_pool(name="outp", bufs=2) as outp,
            tc.tile_pool(name="oall_p", bufs=2) as oall_p,
            tc.tile_pool(name="scp", bufs=2, space="PSUM") as scp,
            tc.tile_pool(name="pvp", bufs=1, space="PSUM") as pvp,
            tc.tile_pool(name="psC", bufs=2, space="PSUM") as psC,
            tc.tile_pool(name="dram", bufs=1, space="DRAM") as dram,
        ):
            # ---- constants ----
            wq_sb = consts.tile([128, 8, EC], BF16)
            wk_sb = consts.tile([128, 8, EC], BF16)
            wv_sb = consts.tile([128, 8, EC], BF16)
            nc.sync.dma_start(wq_sb[:], wq_d[:, :, :])
            nc.sync.dma_start(wk_sb[:], wk_d[:, :, :])
            nc.sync.dma_start(wv_sb[:], wv_d[:, :, :])
            cos_sb = consts.tile([128, T], F32)
            sin_sb = consts.tile([128, T], F32)
            rotm_sb = consts.tile([128, 128], BF16)
            tri_sb = consts.tile([128, 128], BF16)
            ident_sb = consts.tile([128, 128], BF16)
            nc.sync.dma_start(cos_sb[:], cos_d[:, :])
            nc.sync.dma_start(sin_sb[:], sin_d[:, :])
            nc.sync.dma_start(rotm_sb[:], rotm_d[:, :])
            nc.sync.dma_start(tri_sb[:], tri_d[:, :])
            nc.sync.dma_start(ident_sb[:], id_d[:, :])
            wo_sb = consts.tile([128, 8, D], BF16)
            nc.sync.dma_start(wo_sb[:], wo_d[:, :, :])

            # ---- persistent tensors ----
            qT = big.tile([128, NT], BF16, tag="qT")
            kT = big.tile([128, NT], BF16, tag="kT")
            vext = big.tile([128, HPC * B, NBB, 65], BF16, tag="vext")
            nc.vector.memset(vext[:, :, :, 64], 1.0)

            # two collectives per batch: half hf covers tk blocks 8*hf..8*hf+7
            a2a_in = [[dram.tile([N_CORES, 128, 128], BF16, tag=f"ai{b}{hf}",
                                 name=f"ai{b}{hf}") for hf in range(2)]
                      for b in range(B)]
            a2a_out = [[dram.tile([N_CORES, 128, 128], BF16, tag=f"ao{b}{hf}",
                                  name=f"ao{b}{hf}") for hf in range(2)]
                       for b in range(B)]

            xts = {}

            def xt_load(ci):
                xts[ci] = xp.tile([128, 8, 512], BF16, tag="x",
                                  name=f"xt{ci}")
                nc.scalar.dma_start(xts[ci][:], xt_d[:, :, 512 * ci:
                                                     512 * ci + 512])

            # ---------- phase-1 chunk as a list of closures -------------
            def chunk_pieces(ci):
                t0 = 512 * ci
                bb = t0 // T
                s0 = t0 % T
                ps = []
                st = {}

                def proj_mm(w_sb, nm, ko):
                    def f():
                        if ko == 0:
                            st[nm] = psC.tile([128, 512], F32, tag="pp",
                                              name="p" + nm)
                        nc.tensor.matmul(st[nm], w_sb[:, ko, :],
                                         xts[ci][:, ko, :],
                                         start=(ko == 0), stop=(ko == 7))
                    return f

                def drain(nm):
                    def f():
                        st["raw" + nm] = stage.tile([128, 512], BF16,
                                                    tag="raw" + nm,
                                                    name="raw" + nm)
                        nc.scalar.copy(st["raw" + nm][:], st[nm])
                    return f

                def rot(nm):
                    def f():
                        st["rot" + nm] = psC.tile([128, 512], F32, tag="pp",
                                                  name="rot" + nm)
                        nc.tensor.matmul(st["rot" + nm], rotm_sb[:],
                                         st["raw" + nm][:],
                                         start=True, stop=True)
                    return f

                def comb(nm, dest):
                    def f1():
                        st["t2" + nm] = stage.tile([128, 512], F32,
                                                   tag="t2" + nm,
                                                   name="t2" + nm)
                        nc.vector.tensor_tensor(
                            st["t2" + nm][:], st["rot" + nm],
                            sin_sb[:, s0:s0 + 512], mybir.AluOpType.mult)

                    def f2():
                        st["t1" + nm] = stage.tile([128, 512], F32,
                                                   tag="t1" + nm,
                                                   name="t1" + nm)
                        nc.vector.tensor_tensor(
                            st["t1" + nm][:], st["raw" + nm][:],
                            cos_sb[:, s0:s0 + 512], mybir.AluOpType.mult)

                    def f3():
                        nc.vector.tensor_tensor(
                            dest[:, t0:t0 + 512], st["t1" + nm][:],
                            st["t2" + nm][:], mybir.AluOpType.add)
                    return [f1, f2, f3]

                def vtrans(h, bi):
                    def f():
                        pair = bb * HPC + h
                        jg = s0 // 128 + bi
                        tp = psC.tile([128, 512], F32, tag="pp",
                                      name="vtr").bitcast(BF16)[:, 0:64]
                        nc.tensor.transpose(
                            tp, st["rawv"][64 * h:64 * h + 64,
                                           128 * bi:128 * bi + 128],
                            ident_sb[64 * h:64 * h + 64,
                                     64 * h:64 * h + 64])
                        nc.vector.tensor_copy(vext[:, pair, jg, 0:64], tp)
                    return f

                for ko in range(8):
                    ps.append(proj_mm(wq_sb, "q", ko))
                ps.append(drain("q"))
                for ko in range(8):
                    ps.append(proj_mm(wk_sb, "k", ko))
                ps.append(rot("q"))
                ps.extend(comb("q", qT))
                ps.append(drain("k"))
                for ko in range(8):
                    ps.append(proj_mm(wv_sb, "v", ko))
                ps.append(rot("k"))
                ps.extend(comb("k", kT))
                ps.append(drain("v"))
                for h in range(HPC):
                    for bi in range(4):
                        ps.append(vtrans(h, bi))
                return ps

            # ---------- output projection for one half-batch -------------
            def oproj_pieces(bb, hf):
                # tokens: tk blocks {c + 8*hf} -> y rows 256*bb + 128*hf ..
                ps = []
                st = {}

                def load():
                    st["oall"] = oall_p.tile([128, 8, 128], BF16,
                                             tag=f"oall{hf}",
                                             name=f"oall{bb}{hf}")
                    nc.sync.dma_start(
                        st["oall"][:],
                        a2a_out[bb][hf][:].rearrange("s p t -> p s t"))

                def piece(eo):
                    def f():
                        ot = psC.tile([128, 512], F32, tag="pp", name="ot")
                        for ec in range(8):
                            nc.tensor.matmul(
                                ot, st["oall"][:, ec, :],
                                wo_sb[:, ec, 512 * eo:512 * eo + 512],
                                start=(ec == 0), stop=(ec == 7))
                        ys = outp.tile([128, 512], F32, tag="ys", name="ys")
                        nc.vector.tensor_copy(ys[:], ot)
                        nc.sync.dma_start(
                            y_d[256 * bb + 128 * hf:256 * bb + 128 * hf + 128,
                                512 * eo:512 * eo + 512], ys[:])
                    return f

                ps.append(load)
                for eo in range(2):
                    ps.append(piece(eo))
                return ps

            # ---------- attention for one batch, with filler -------------
            def do_attn(bb, filler):
                fidx = [0]

                def pop_filler(k):
                    while k > 0 and fidx[0] < len(filler):
                        filler[fidx[0]]()
                        fidx[0] += 1
                        k -= 1

                tb0 = bb * T
                pair0 = bb * HPC
                deferred = []
                pending = []
                steps_left = [40 + 1]

                def pop_adaptive():
                    steps_left[0] = max(1, steps_left[0] - 1)
                    k = -(-(len(filler) - fidx[0]) // steps_left[0])
                    pop_filler(max(2, k))

                for half in range(2):
                    unns = []
                    for q4 in (2 * half, 2 * half + 1):
                        if q4 == 3 and deferred:
                            filler.extend(deferred)
                            deferred = []
                        jmax = 4 * q4 + 4
                        tq0 = tb0 + 512 * q4
                        pvt = [pvp.tile([65, 512], F32, tag=f"pv{hh}",
                                        name=f"pvt{hh}") for hh in range(2)]

                        def scores(j):
                            sc = scp.tile([128, 2, 512], F32, tag="sc",
                                          name="sc")
                            lo = max(0, 128 * j - 512 * q4)
                            for hh in range(2):
                                nc.tensor.matmul(
                                    sc[:, hh, lo:512],
                                    kT[64 * hh:64 * hh + 64,
                                       tb0 + 128 * j:tb0 + 128 * j + 128],
                                    qT[64 * hh:64 * hh + 64,
                                       tq0 + lo:tq0 + 512],
                                    start=True, stop=True)
                            return sc, lo

                        s_cur = scores(0)
                        prev = None
                        for j in range(jmax):
                            sc, lo = s_cur
                            ex = expp.tile([128, 2, 512], BF16, tag="ex",
                                           name="ex")
                            nc.scalar.activation(
                                ex[:, :, lo:512], sc[:, :, lo:512],
                                mybir.ActivationFunctionType.Exp,
                                scale=0.125)
                            if prev is not None:
                                pj, plo, pex = prev
                                for hh in range(2):
                                    nc.tensor.matmul(
                                        pvt[hh][:, plo:512],
                                        vext[:, pair0 + hh, pj, 0:65],
                                        pex[:, hh, plo:512],
                                        start=(pj == 0), stop=False)
                            if j + 1 < jmax:
                                s_cur = scores(j + 1)
                            if j >= 4 * q4:
                                d0 = 128 * (j - 4 * q4)
                                for hh in range(2):
                                    nc.vector.tensor_tensor(
                                        ex[:, hh, d0:d0 + 128],
                                        ex[:, hh, d0:d0 + 128],
                                        tri_sb[:], mybir.AluOpType.mult)
                            prev = (j, lo, ex)
                            if j == 1 and pending:
                                for fn in pending:
                                    fn()
                                pending = []
                            pop_adaptive()
                        pj, plo, pex = prev
                        for hh in range(2):
                            nc.tensor.matmul(
                                pvt[hh][:, plo:512],
                                vext[:, pair0 + hh, pj, 0:65],
                                pex[:, hh, plo:512],
                                start=(pj == 0), stop=True)
                        # drain PSUM per head so the next quarter's PV can
                        # start as soon as possible
                        unn = outp.tile([65, 2, 512], F32, tag="unn",
                                        name="unn")
                        for hh in range(2):
                            nc.vector.tensor_copy(unn[:, hh, :], pvt[hh][:])
                        unns.append((q4, unn))
                        pop_filler(4)

                    # ---- batched normalize + ship for this half ----
                    # ln then exp(-x) in place on the denominator rows,
                    # grouped so the ACT table only swaps twice per half;
                    # hoisted into the next quarter's j-loop so the next
                    # exp isn't stuck behind it on the ACT queue.
                    def norm_ship(unns=unns, half=half):
                        # 1/denom off the ACT queue: scatter the 1024 denoms
                        # across partitions, accurate DVE reciprocal on an
                        # 8-wide free dim, gather back.  The permutation is
                        # irrelevant (elementwise op).
                        for idx, (q4, unn) in enumerate(unns):
                            dsc = outp.tile([128, 8], F32,
                                            tag=f"dsc{idx}", name="dsc")
                            nc.sync.dma_start(dsc[:], unn[64:65, :, :])
                            dsr = outp.tile([128, 8], F32,
                                            tag=f"dsr{idx}", name="dsr")
                            nc.vector.reciprocal(dsr[:], dsc[:])
                            unns[idx] = (q4, unn, dsr)
                        for idx, (q4, unn, dsr) in enumerate(unns):
                            rec = outp.tile([1, 2, 512], F32,
                                            tag=f"rec{idx}", name="rec")
                            nc.sync.dma_start(rec[:], dsr[:])
                            recb = outp.tile([64, 2, 512], F32,
                                             tag=f"recb{idx}", name="recb")
                            nc.gpsimd.partition_broadcast(recb[:], rec[:])
                            ao = outp.tile([64, 2, 512], BF16,
                                           tag=f"aot{idx}", name="aot")
                            nc.vector.scalar_tensor_tensor(
                                ao[:], unn[0:64, :, :], 1.0, recb[:],
                                mybir.AluOpType.mult, mybir.AluOpType.mult)
                            for hh in range(2):
                                for tb in range(4):
                                    j16 = 4 * q4 + tb
                                    dest = j16 % 8
                                    hfi = j16 // 8
                                    nc.sync.dma_start(
                                        a2a_in[bb][hfi][
                                            dest, 64 * hh:64 * hh + 64, :],
                                        ao[:, hh, 128 * tb:128 * tb + 128])
                        nc.gpsimd.collective_compute(
                            "AllToAll", mybir.AluOpType.bypass,
                            replica_groups=[list(range(N_CORES))],
                            ins=[a2a_in[bb][half].opt()],
                            outs=[a2a_out[bb][half].opt()])
                        if bb == B - 1:
                            if half == 0:
                                deferred.extend(oproj_pieces(bb, half))
                            else:
                                filler.extend(oproj_pieces(bb, half))

                    if half == 0:
                        pending.append(norm_ship)
                    else:
                        norm_ship()
                pop_filler(len(filler))

            # ================= main schedule ==========================
            xt_load(0)
            xt_load(1)
            for p in chunk_pieces(0):
                p()
            for bb in range(B):
                filler = []
                lo_ci = 1 if bb == 0 else 4 * bb + 4
                hi_ci = min(16, 4 * bb + 8)
                for ci in range(lo_ci, hi_ci):
                    filler.extend(chunk_pieces(ci))
                    if ci + 1 < 16:
                        filler.append(lambda c=ci + 1: xt_load(c))
                if bb - 1 >= 0:
                    for hf in range(2):
                        filler.extend(oproj_pieces(bb - 1, hf))
                do_attn(bb, filler)

    nc.compile()
    return nc


def _host_inputs(x, Wq, Wk, Wv, Wo, token_positions):
    """Per-core in_maps with transposed/tiled layouts."""
    x = np.asarray(x, dtype=np.float32)
    xt_bf = np.ascontiguousarray(
        x.reshape(NT, D).T.reshape(8, 128, NT).transpose(1, 0, 2)
    ).astype(ml_dtypes.bfloat16)

    pos = np.asarray(token_positions).astype(np.float64)
    inv_freq = 1.0 / (THETA ** (np.arange(0, DH, 2, dtype=np.float64) / DH))
    ang = pos[None, :] * inv_freq[:, None]          # [32, T]
    cos_p = np.cos(ang)
    sin_p = np.sin(ang)
    d_idx = (np.arange(128) % 64) // 2
    cosb = cos_p[d_idx, :].astype(np.float32)
    sinb = sin_p[d_idx, :].astype(np.float32)

    rotm = np.zeros((128, 128), dtype=np.float32)
    for i in range(64):
        rotm[2 * i + 1, 2 * i] = -1.0
        rotm[2 * i, 2 * i + 1] = 1.0
    rotm = rotm.astype(ml_dtypes.bfloat16)
    tri = np.tril(np.ones((128, 128), dtype=np.float32)).T  # [tk, tq]
    tri = tri.astype(ml_dtypes.bfloat16)
    identb = np.eye(128, dtype=np.float32).astype(ml_dtypes.bfloat16)

    def wtiles(W, sl):
        Wt = np.ascontiguousarray(W[sl, :].T)        # [D, e]
        return np.ascontiguousarray(
            Wt.reshape(8, 128, Wt.shape[1]).transpose(1, 0, 2))

    WoT = np.ascontiguousarray(np.asarray(Wo, dtype=np.float32).T)
    wo_t = np.ascontiguousarray(WoT.reshape(8, 128, D).transpose(1, 0, 2))

    in_maps = []
    for c in range(N_CORES):
        sl = slice(EC * c, EC * (c + 1))
        in_maps.append({
            "xt": xt_bf,
            "wq": wtiles(np.asarray(Wq, np.float32), sl).astype(
                ml_dtypes.bfloat16),
            "wk": wtiles(np.asarray(Wk, np.float32), sl).astype(
                ml_dtypes.bfloat16),
            "wv": wtiles(np.asarray(Wv, np.float32), sl).astype(
                ml_dtypes.bfloat16),
            "wo": wo_t.astype(ml_dtypes.bfloat16),
            "cosb": cosb,
            "sinb": sinb,
            "rotm": rotm,
            "trimask": tri,
            "identb": identb,
        })
    return in_maps


def kernel(x, Wq, Wk, Wv, Wo, token_positions):
    global last_results
    if "nc" not in _CACHE:
        _CACHE["nc"] = _build_program()
    nc = _CACHE["nc"]
    in_maps = _host_inputs(x, Wq, Wk, Wv, Wo, token_positions)
    res = bass_utils.run_bass_kernel_spmd(nc, in_maps, list(range(N_CORES)))
    last_results = res
    y = np.empty((NT, D), dtype=np.float32)
    for c in range(N_CORES):
        yc = res.results[c]["y"]
        for bb in range(B):
            for hf in range(2):
                g0 = 2048 * bb + 128 * (c + 8 * hf)
                y[g0:g0 + 128] = yc[256 * bb + 128 * hf:
                                    256 * bb + 128 * hf + 128]
    return y.reshape(B, T, D)
